# revision 5
# baseline (speedup 1.0000x reference)
"""Trainium2 Bass kernel for causal multi-head attention.

Problem: B=4, S=2048, D=1024, H=16 heads, Dh=64, fp32, causal mask.
Sharding: 8 cores = 4 batches x 2 head-groups (8 heads each). No
collectives: each core produces a partial output projection y_T
[1024, 2048] for its batch; the host sums the two head-group partials
per batch and adds the output bias.

Device-side design (per core):
  - QKV projections in fp32r (full PE rate at moving dim >= 256):
      Q_T, K_T = W_slice^T.T @ X_T      (lhsT = W^T slices, rhs = X_T)
      V natural [token, feature]        (lhsT = X_T chunk, rhs = W_v^T)
    V is stored bf16 with a 65th ones-column per head.
  - scores transposed per head: S_T[k, q] = K_T_h.T @ Q_T_h (contraction
    over Dh=64 partitions), fp32r, in 256-wide q sub-blocks; exp on ACT
    (scale folded) emits bf16 probabilities p2.
  - attnV FLIPPED: out[q_part, dh] = p2_slice.T @ V_tile — moving dim is
    only 65 (64 V cols + ones col), vs 512 in the [dh, q] orientation,
    halving attnV PE cycles (matmul cost is moving-dim rows only; output
    partitions are free). Requires bf16 operands (fp32r is 1/4 rate
    below 256-wide moving). The ones column accumulates the softmax
    denominator per q PARTITION, so the division is a cheap per-partition
    DVE tensor_scalar instead of ones-matmul broadcasts.
  - AO[q, c] is transposed back to AO_T[c, q] with bf16 PE transposes
    (128 rows each) to feed the output projection (contraction over c
    needs c on partitions); projection runs in bf16 (same PE rate).
  - causal: dead k-tiles skipped per 128-wide q sub-tile; only the
    128-wide diagonal band is masked with one DVE multiply per k-tile
    (band-local f>=p triangle identical for every diagonal tile).
  - cross-phase software pipelining: next q-block's QKV groups, previous
    blocks' projection groups, and AO transposes are woven between
    attention steps so the in-order PE stream always has independent
    work during softmax (ACT) dependency stalls.
"""

import numpy as np
import ml_dtypes

import concourse.tile as tile
from concourse import bacc, mybir
from concourse.bass_utils import run_bass_kernel_spmd

B = 4
S = 2048
D = 1024
H = 16
DH = 64
NCORES = 8
HPC = 8  # heads per core
C = HPC * DH  # 512 local channels per core
QB = 512  # QKV/projection block (matmul moving free dim)
AB = 256  # attention q sub-block
NQB = S // QB  # 4
NAB = S // AB  # 8
NKT = S // 128  # 16 k-tiles
SCALE = 1.0 / float(np.sqrt(DH))

F32 = mybir.dt.float32
BF = mybir.dt.bfloat16
AF = mybir.ActivationFunctionType
ALU = mybir.AluOpType


def build_nc():
    """Build the single-core Bass program (SPMD-replicated on 8 cores)."""
    MDT = mybir.dt.float32r

    nc = bacc.Bacc("TRN2", target_bir_lowering=False, debug=False)
    regions = []
    nc._regions = regions

    def region(name):
        regions.append((name, len(nc.inst_map)))

    xt = nc.dram_tensor("xt", [D, S], MDT, kind="ExternalInput").ap()
    wqt = nc.dram_tensor("wqt", [D, C], MDT, kind="ExternalInput").ap()
    wkt = nc.dram_tensor("wkt", [D, C], MDT, kind="ExternalInput").ap()
    wvt = nc.dram_tensor("wvt", [D, C], MDT, kind="ExternalInput").ap()
    wot = nc.dram_tensor("wot", [C, D], BF, kind="ExternalInput").ap()
    bq_d = nc.dram_tensor("bq", [128, C // 128], F32, kind="ExternalInput").ap()
    bk_d = nc.dram_tensor("bk", [128, C // 128], F32, kind="ExternalInput").ap()
    bvb_d = nc.dram_tensor("bvb", [128, C], F32, kind="ExternalInput").ap()
    yt = nc.dram_tensor("yt", [D, S], F32, kind="ExternalOutput").ap()

    xt_r = xt.rearrange("(mt p) s -> p mt s", p=128)

    with tile.TileContext(nc) as tc:
        with (
            tc.tile_pool(name="singles", bufs=1) as singles,
            tc.tile_pool(name="xtp", bufs=1) as xtp,
            tc.tile_pool(name="qtp", bufs=2) as qtp,
            tc.tile_pool(name="p2p", bufs=3) as p2p,
            tc.tile_pool(name="aop", bufs=3) as aop,
            tc.tile_pool(name="atp", bufs=2) as atp,
            tc.tile_pool(name="rp", bufs=2) as rp,
            tc.tile_pool(name="yp", bufs=4) as yp,
            tc.tile_pool(name="ps_mm", bufs=2, space="PSUM") as ps_mm,
            tc.tile_pool(name="ps_s", bufs=2, space="PSUM") as ps_s_pool,
            tc.tile_pool(name="ps_av", bufs=2, space="PSUM") as ps_av_pool,
            tc.tile_pool(name="ps_tr", bufs=2, space="PSUM") as ps_tr_pool,
        ):
            # ---- persistent tiles -------------------------------------
            w_q = singles.tile([128, 8, C], MDT, tag="w_q")
            w_k = singles.tile([128, 8, C], MDT, tag="w_k")
            w_v = singles.tile([128, 8, C], MDT, tag="w_v")
            w_o = singles.tile([128, 4, D], BF, tag="w_o")
            bq_sb = singles.tile([128, C // 128], F32, tag="bq")
            bk_sb = singles.tile([128, C // 128], F32, tag="bk")
            bvb_sb = singles.tile([128, C], F32, tag="bvb")
            kt_sb = singles.tile([128, 4, S], MDT, tag="kt")
            v_sb = singles.tile([128, NKT, HPC, DH + 1], BF, tag="v")
            masks = singles.tile([128, 2, QB], BF, tag="masks")
            ident = singles.tile([128, 128], BF, tag="ident")

            # first x block goes ahead of everything so PE unblocks ASAP;
            # wq follows j-tile-major so the first Q group only needs the
            # jt=0 quarter of wq
            xt_cur = xtp.tile([128, 8, QB], MDT, tag="xt")
            wq_r = wqt.rearrange("(mt p) j -> p mt j", p=128)
            wk_r = wkt.rearrange("(mt p) j -> p mt j", p=128)
            wv_r = wvt.rearrange("(mt p) j -> p mt j", p=128)
            for mt in range(8):
                nc.sync.dma_start(xt_cur[:, mt, :], xt_r[:, mt, 0:QB])
                nc.sync.dma_start(
                    w_q[:, mt, 0:128], wq_r[:, mt, 0:128]
                )
            for jt in range(1, 4):
                js = slice(jt * 128, (jt + 1) * 128)
                for mt in range(8):
                    nc.sync.dma_start(w_q[:, mt, js], wq_r[:, mt, js])
            for jt in range(4):
                js = slice(jt * 128, (jt + 1) * 128)
                for mt in range(8):
                    nc.sync.dma_start(w_k[:, mt, js], wk_r[:, mt, js])
            for mt in range(8):
                nc.sync.dma_start(w_v[:, mt, :], wv_r[:, mt, :])
            # small/constant inputs ride the idle gpsimd (SWDGE) queue
            nc.gpsimd.dma_start(bq_sb, bq_d)
            nc.gpsimd.dma_start(bk_sb, bk_d)
            nc.gpsimd.dma_start(bvb_sb, bvb_d)

            # ones column (65th) of every per-head V block; bf16 memsets
            # are encodable so no host ones tensor is needed
            nc.vector.memset(v_sb[:, :, :, DH : DH + 1], 1.0)
            # mask tile; only the [128:256] slice of row 0 is used — in
            # band-local coordinates it is the f>=p triangle that every
            # diagonal tile needs
            nc.vector.memset(masks, 1.0)
            # warm-up matmuls on the freshly-memset mask tile: they depend
            # only on the early DVE memset, so they execute during the
            # initial DMA wait and keep the PE activity window warm
            for _ in range(5):
                ps_w = ps_mm.tile([128, QB], F32, tag="mm")
                nc.tensor.matmul(
                    ps_w, masks[:, 0, 0:128], masks[:, 1, :], start=True, stop=True
                )
            nc.gpsimd.affine_select(
                out=masks,
                in_=masks,
                compare_op=ALU.is_ge,
                fill=0.0,
                base=-128,
                pattern=[[-256, 2], [1, QB]],
                channel_multiplier=-1,
            )
            # identity for PE transposes
            nc.gpsimd.memset(ident, 0.0)
            nc.gpsimd.affine_select(
                out=ident,
                in_=ident,
                compare_op=ALU.not_equal,
                fill=1.0,
                base=0,
                pattern=[[-1, 128]],
                channel_multiplier=1,
            )
            bvb_r = bvb_sb.rearrange("p (h d) -> p h d", d=DH)

            def emit_qkv_group(qb2, xt_b, qt_b, kind, idx):
                """One psum accumulation group of a QKV projection phase.

                kind 'q'/'k': output j-tile idx of Q_T/K_T; kind 'v': seq
                chunk idx of V."""
                qs2 = slice(qb2 * QB, (qb2 + 1) * QB)
                ps = ps_mm.tile([128, QB], F32, tag="mm")
                if kind in ("q", "k"):
                    w_sb, b_sb = (w_q, bq_sb) if kind == "q" else (w_k, bk_sb)
                    jt = idx
                    for mt in range(8):
                        nc.tensor.matmul(
                            ps,
                            w_sb[:, mt, jt * 128 : (jt + 1) * 128],
                            xt_b[:, mt, :],
                            start=(mt == 0),
                            stop=(mt == 7),
                        )
                    dst = qt_b[:, jt, :] if kind == "q" else kt_sb[:, jt, qs2]
                    nc.vector.tensor_scalar_add(dst, ps, b_sb[:, jt : jt + 1])
                else:
                    kc = idx
                    for mt in range(8):
                        nc.tensor.matmul(
                            ps,
                            xt_b[:, mt, kc * 128 : (kc + 1) * 128],
                            w_v[:, mt, :],
                            start=(mt == 0),
                            stop=(mt == 7),
                        )
                    with nc.allow_low_precision(
                        reason="V in bf16 feeds bf16 attnV matmuls; ~0.4%"
                        " rel err is within the 2e-2 tolerance"
                    ):
                        nc.vector.tensor_tensor(
                            v_sb[:, qb2 * 4 + kc, :, 0:DH],
                            ps.rearrange("p (h d) -> p h d", d=DH),
                            bvb_r,
                            ALU.add,
                        )

            GROUPS = [("q", i) for i in range(4)] + [("k", i) for i in range(4)] + [
                ("v", i) for i in range(4)
            ]

            def make_proj_group(qb2, ao_t_b, et, on_act=False):
                qs2 = slice(qb2 * QB, (qb2 + 1) * QB)

                def emit():
                    ps = ps_mm.tile([128, QB], F32, tag="mm")
                    for ct in range(4):
                        nc.tensor.matmul(
                            ps,
                            w_o[:, ct, et * 128 : (et + 1) * 128],
                            ao_t_b[:, ct, :],
                            start=(ct == 0),
                            stop=(ct == 3),
                        )
                    y_t = yp.tile([128, QB], F32, tag="y")
                    if on_act:
                        nc.scalar.activation(y_t, ps, AF.Copy)
                    else:
                        nc.vector.tensor_copy(y_t, ps)
                    nc.sync.dma_start(yt[et * 128 : (et + 1) * 128, qs2], y_t)

                return emit

            def make_tr_group(ao_b, ao_t_b, hp, base):
                """Transpose a pair's AO [q, 2x64] sub-tiles back to
                AO_T[c, q] for the output projection."""

                def emit():
                    ps_t = ps_tr_pool.tile([128, 2, 128], BF, tag="tr")
                    for qsub in range(2):
                        nc.tensor.transpose(
                            ps_t[:, qsub, :], ao_b[:, qsub], ident
                        )
                    nc.vector.tensor_copy(
                        ao_t_b[:, hp, base : base + AB],
                        ps_t.rearrange("p a q -> p (a q)"),
                    )

                return emit

            workq = []  # FIFO: transpose groups, then projection groups
            next_groups = []

            # q-block 0 projections up front
            region("qkv0")
            qt_blk = qtp.tile([128, 4, QB], MDT, tag="qt")
            for kind, idx in GROUPS:
                emit_qkv_group(0, xt_cur, qt_blk, kind, idx)
            xt_blk = xt_cur
            ao_t_blk = atp.tile([128, 4, QB], BF, tag="aot")

            for qb in range(NQB):
                # stage next q-block: x prefetch + Q_T tile; its 12
                # projection groups are woven between attention steps below
                if qb + 1 < NQB:
                    xt_next = xtp.tile([128, 8, QB], MDT, tag="xt")
                    nqs = slice((qb + 1) * QB, (qb + 2) * QB)
                    for mt in range(8):
                        nc.sync.dma_start(xt_next[:, mt, :], xt_r[:, mt, nqs])
                    qt_next = qtp.tile([128, 4, QB], MDT, tag="qt")
                    ao_t_next = atp.tile([128, 4, QB], BF, tag="aot")
                    next_groups = list(GROUPS)
                else:
                    xt_next = qt_next = ao_t_next = None
                    next_groups = []
                if qb == 0:
                    # Wo is first needed by proj0, well after qb1's x
                    # prefetch — keep it behind that in the load queue
                    wo_r = wot.rearrange("(ct p) e -> p ct e", p=128)
                    for ct in range(4):
                        nc.sync.dma_start(w_o[:, ct, :], wo_r[:, ct, :])

                for sub in range(2):
                    a = 2 * qb + sub  # attention block index (256 tokens)
                    qoff = sub * AB
                    region(f"attn{a}")
                    for hp in range(4):
                        # per-pair filler budget: transposes/projections
                        # first (FIFO), then next block's QKV groups
                        filler = []
                        for _ in range(2):
                            if workq:
                                filler.append(workq.pop(0))
                        for _ in range(3):
                            if next_groups:
                                kind, idx = next_groups.pop(0)
                                filler.append(
                                    lambda k=kind, i=idx, q2=qb + 1, xn=xt_next,
                                    qn=qt_next: emit_qkv_group(q2, xn, qn, k, i)
                                )
                            elif workq:
                                filler.append(workq.pop(0))

                        ps_av = ps_av_pool.tile([128, 2, 2, DH + 1], F32, tag="av")
                        n_kt = 2 * a + 2
                        for kt in range(n_kt):
                            if kt % 3 == 2 and kt != n_kt - 1 and filler:
                                filler.pop(0)()
                            kts = slice(kt * 128, (kt + 1) * 128)
                            ps_sc = ps_s_pool.tile([128, 2, AB], F32, tag="s")
                            for h in range(2):
                                hs = slice(h * 64, h * 64 + 64)
                                nc.tensor.matmul(
                                    ps_sc[:, h, :],
                                    kt_sb[hs, hp, kts],
                                    qt_blk[hs, hp, qoff : qoff + AB],
                                    start=True,
                                    stop=True,
                                )
                            p2 = p2p.tile([128, 2, AB], BF, tag="p")
                            nc.scalar.activation(p2, ps_sc, AF.Exp, scale=SCALE)
                            r = kt - 2 * a  # 0 for first diag kt, 1 for second
                            if r >= 0:
                                band = slice(r * 128, r * 128 + 128)
                                nc.vector.tensor_tensor(
                                    p2[:, :, band],
                                    p2[:, :, band],
                                    masks[:, 0, None, 128:256].to_broadcast(
                                        (128, 2, 128)
                                    ),
                                    ALU.mult,
                                )
                            # flipped attnV: per live 128-wide q sub-tile
                            for qsub in range(2):
                                if 2 * a + qsub < kt:
                                    continue  # q sub-tile entirely above diag
                                for h in range(2):
                                    nc.tensor.matmul(
                                        ps_av[:, qsub, h, :],
                                        p2[:, h, qsub * 128 : qsub * 128 + 128],
                                        v_sb[:, kt, 2 * hp + h, :],
                                        start=(kt == 0),
                                        stop=(kt == 2 * a + qsub),
                                    )

                        # softmax denominators live in psum column 64 per q
                        # partition: reciprocal + per-partition scale
                        r2 = rp.tile([128, 2, 2, 1], F32, tag="r2")
                        nc.vector.reciprocal(r2, ps_av[:, :, :, DH : DH + 1])
                        if filler:
                            filler.pop(0)()
                        ao_b = aop.tile([128, 2, 2, DH], BF, tag="ao")
                        with nc.allow_low_precision(
                            reason="AO in bf16 feeds bf16 projection matmuls;"
                            " ~0.4% rel err is within the 2e-2 tolerance"
                        ):
                            for qsub in range(2):
                                for h in range(2):
                                    nc.vector.tensor_scalar_mul(
                                        ao_b[:, qsub, h, :],
                                        ps_av[:, qsub, h, 0:DH],
                                        r2[:, qsub, h, :],
                                    )
                        workq.append(
                            make_tr_group(ao_b, ao_t_blk, hp, qoff)
                        )
                        while filler:
                            filler.pop(0)()

                # all 8 pairs of this 512-block done: queue its projection
                workq.extend(
                    make_proj_group(qb, ao_t_blk, et, on_act=(qb == NQB - 1))
                    for et in range(8)
                )
                xt_blk = xt_next
                qt_blk = qt_next
                ao_t_blk = ao_t_next

            # drain the last q-block's transposes + projection
            region("proj3")
            while workq:
                workq.pop(0)()

    nc.compile()
    return nc


def make_in_maps(x, Wq_w, Wk_w, Wv_w, Wo_w, Wq_b, Wk_b, Wv_b):
    """Per-core host-side sharding + layout prep."""
    x = np.asarray(x, dtype=np.float32)
    in_maps = []
    for c in range(NCORES):
        b, g = divmod(c, 2)
        cols = slice(g * C, (g + 1) * C)
        in_maps.append(
            {
                "xt": np.ascontiguousarray(x[b].T),
                "wqt": np.ascontiguousarray(np.asarray(Wq_w).T[:, cols]),
                "wkt": np.ascontiguousarray(np.asarray(Wk_w).T[:, cols]),
                "wvt": np.ascontiguousarray(np.asarray(Wv_w).T[:, cols]),
                "wot": np.ascontiguousarray(np.asarray(Wo_w)[:, cols].T).astype(
                    ml_dtypes.bfloat16
                ),
                "bq": np.ascontiguousarray(
                    np.asarray(Wq_b)[cols].reshape(C // 128, 128).T
                ),
                "bk": np.ascontiguousarray(
                    np.asarray(Wk_b)[cols].reshape(C // 128, 128).T
                ),
                "bvb": np.ascontiguousarray(
                    np.tile(np.asarray(Wv_b)[cols][None, :], (128, 1))
                ),
            }
        )
    return in_maps


_NC_CACHE = {}
last_results = None  # test harness reads profiling info from here


def kernel(x, mask, Wq_w, Wq_b, Wk_w, Wk_b, Wv_w, Wv_b, Wo_w, Wo_b):
    global last_results
    if "nc" not in _NC_CACHE:
        _NC_CACHE["nc"] = build_nc()
    nc = _NC_CACHE["nc"]

    in_maps = make_in_maps(x, Wq_w, Wk_w, Wv_w, Wo_w, Wq_b, Wk_b, Wv_b)
    res = run_bass_kernel_spmd(nc, in_maps, list(range(NCORES)))
    last_results = res

    bo = np.asarray(Wo_b, dtype=np.float32)
    y = np.empty((B, S, D), dtype=np.float32)
    for b in range(B):
        yt = res.results[2 * b]["yt"] + res.results[2 * b + 1]["yt"]
        y[b] = yt.T + bo[None, :]
    return y


# revision 14
# speedup vs baseline: 1.1063x; 1.1063x over previous
"""Trainium2 Bass kernel for causal multi-head attention.

Problem: B=4, S=2048, D=1024, H=16 heads, Dh=64, fp32, causal mask.
Sharding: 8 cores = 4 batches x 2 head-groups (8 heads each). No
collectives: each core produces a partial output projection y_T
[1024, 2048] for its batch; the host sums the two head-group partials
per batch and adds the output bias.

Device-side design (per core):
  - QKV projections in fp32r (full PE rate at moving dim >= 256):
      Q_T, K_T = W_slice^T.T @ X_T      (lhsT = W^T slices, rhs = X_T)
      V natural [token, feature]        (lhsT = X_T chunk, rhs = W_v^T)
    V is stored bf16 with a 65th ones-column per head.
  - scores transposed per head: S_T[k, q] = K_T_h.T @ Q_T_h (contraction
    over Dh=64 partitions), fp32r, in 256-wide q sub-blocks; exp on ACT
    (scale folded) emits bf16 probabilities p2.
  - attnV FLIPPED: out[q_part, dh] = p2_slice.T @ V_tile — moving dim is
    only 65 (64 V cols + ones col), vs 512 in the [dh, q] orientation,
    halving attnV PE cycles (matmul cost is moving-dim rows only; output
    partitions are free). Requires bf16 operands (fp32r is 1/4 rate
    below 256-wide moving). The ones column accumulates the softmax
    denominator per q PARTITION, so the division is a cheap per-partition
    DVE tensor_scalar instead of ones-matmul broadcasts.
  - AO[q, c] is transposed back to AO_T[c, q] with bf16 PE transposes
    (128 rows each) to feed the output projection (contraction over c
    needs c on partitions); projection runs in bf16 (same PE rate).
  - causal: dead k-tiles skipped per 128-wide q sub-tile; only the
    128-wide diagonal band is masked with one DVE multiply per k-tile
    (band-local f>=p triangle identical for every diagonal tile).
  - cross-phase software pipelining: next q-block's QKV groups, previous
    blocks' projection groups, and AO transposes are woven between
    attention steps so the in-order PE stream always has independent
    work during softmax (ACT) dependency stalls.
"""

import numpy as np
import ml_dtypes

import concourse.tile as tile
from concourse import bacc, mybir
from concourse.bass_utils import run_bass_kernel_spmd

B = 4
S = 2048
D = 1024
H = 16
DH = 64
NCORES = 8
HPC = 8  # heads per core
C = HPC * DH  # 512 local channels per core
QB = 512  # QKV/projection block (matmul moving free dim)
AB = 256  # attention q sub-block
NQB = S // QB  # 4
NAB = S // AB  # 8
NKT = S // 128  # 16 k-tiles
SCALE = 1.0 / float(np.sqrt(DH))

F32 = mybir.dt.float32
BF = mybir.dt.bfloat16
AF = mybir.ActivationFunctionType
ALU = mybir.AluOpType


def build_nc():
    """Build the single-core Bass program (SPMD-replicated on 8 cores)."""
    MDT = mybir.dt.float32r

    nc = bacc.Bacc("TRN2", target_bir_lowering=False, debug=False)
    regions = []
    nc._regions = regions

    def region(name):
        regions.append((name, len(nc.inst_map)))

    xt = nc.dram_tensor("xt", [D, S], MDT, kind="ExternalInput").ap()
    wqt = nc.dram_tensor("wqt", [D, C], MDT, kind="ExternalInput").ap()
    wkt = nc.dram_tensor("wkt", [D, C], MDT, kind="ExternalInput").ap()
    wvt = nc.dram_tensor("wvt", [D, C], MDT, kind="ExternalInput").ap()
    wot = nc.dram_tensor("wot", [C, D], BF, kind="ExternalInput").ap()
    bq_d = nc.dram_tensor("bq", [128, C // 128], F32, kind="ExternalInput").ap()
    bk_d = nc.dram_tensor("bk", [128, C // 128], F32, kind="ExternalInput").ap()
    bvb_d = nc.dram_tensor("bvb", [128, C], F32, kind="ExternalInput").ap()
    yt = nc.dram_tensor("yt", [D, S], F32, kind="ExternalOutput").ap()

    xt_r = xt.rearrange("(mt p) s -> p mt s", p=128)

    with tile.TileContext(nc) as tc:
        with (
            tc.tile_pool(name="singles", bufs=1) as singles,
            tc.tile_pool(name="xtp", bufs=2) as xtp,
            tc.tile_pool(name="qtp", bufs=2) as qtp,
            tc.tile_pool(name="p2p", bufs=3) as p2p,
            tc.tile_pool(name="aop", bufs=4) as aop,
            tc.tile_pool(name="atp", bufs=3) as atp,
            tc.tile_pool(name="rp", bufs=2) as rp,
            tc.tile_pool(name="yp", bufs=4) as yp,
            tc.tile_pool(name="ps_mm", bufs=2, space="PSUM") as ps_mm,
            tc.tile_pool(name="ps_s", bufs=2, space="PSUM") as ps_s_pool,
            tc.tile_pool(name="ps_av", bufs=2, space="PSUM") as ps_av_pool,
        ):
            # ---- persistent tiles -------------------------------------
            w_q = singles.tile([128, 8, C], MDT, tag="w_q")
            w_k = singles.tile([128, 8, C], MDT, tag="w_k")
            w_v = singles.tile([128, 8, C], MDT, tag="w_v")
            w_o = singles.tile([128, 4, D], BF, tag="w_o")
            bq_sb = singles.tile([128, C // 128], F32, tag="bq")
            bk_sb = singles.tile([128, C // 128], F32, tag="bk")
            bvb_sb = singles.tile([128, C], F32, tag="bvb")
            kt_sb = singles.tile([128, 4, S], MDT, tag="kt")
            v_sb = singles.tile([128, NKT, HPC, DH + 1], BF, tag="v")
            masks = singles.tile([128, 2, QB], BF, tag="masks")
            ident = singles.tile([128, 128], F32, tag="ident")

            # first x block goes ahead of everything so PE unblocks ASAP;
            # wq follows j-tile-major so the first Q group only needs the
            # jt=0 quarter of wq
            xt_cur = xtp.tile([128, 8, QB], MDT, tag="xt")
            wq_r = wqt.rearrange("(mt p) j -> p mt j", p=128)
            wk_r = wkt.rearrange("(mt p) j -> p mt j", p=128)
            wv_r = wvt.rearrange("(mt p) j -> p mt j", p=128)
            for mt in range(8):
                nc.sync.dma_start(xt_cur[:, mt, :], xt_r[:, mt, 0:QB])
                nc.sync.dma_start(
                    w_q[:, mt, 0:128], wq_r[:, mt, 0:128]
                )
            for jt in range(1, 4):
                js = slice(jt * 128, (jt + 1) * 128)
                for mt in range(8):
                    nc.sync.dma_start(w_q[:, mt, js], wq_r[:, mt, js])
            for jt in range(4):
                js = slice(jt * 128, (jt + 1) * 128)
                for mt in range(8):
                    nc.sync.dma_start(w_k[:, mt, js], wk_r[:, mt, js])
            for mt in range(8):
                nc.sync.dma_start(w_v[:, mt, :], wv_r[:, mt, :])
            # small/constant inputs ride the idle gpsimd (SWDGE) queue
            nc.gpsimd.dma_start(bq_sb, bq_d)
            nc.gpsimd.dma_start(bk_sb, bk_d)
            nc.gpsimd.dma_start(bvb_sb, bvb_d)

            # ones column (65th) of every per-head V block; bf16 memsets
            # are encodable so no host ones tensor is needed
            nc.vector.memset(v_sb[:, :, :, DH : DH + 1], 1.0)
            # mask tile; only the [128:256] slice of row 0 is used — in
            # band-local coordinates it is the f>=p triangle that every
            # diagonal tile needs
            nc.vector.memset(masks, 1.0)
            # warm-up matmuls on the freshly-memset mask tile: they depend
            # only on the early DVE memset, so they execute during the
            # initial DMA wait and keep the PE activity window warm
            for _ in range(5):
                ps_w = ps_mm.tile([128, QB], F32, tag="mm")
                nc.tensor.matmul(
                    ps_w, masks[:, 0, 0:128], masks[:, 1, :], start=True, stop=True
                )
            nc.gpsimd.affine_select(
                out=masks,
                in_=masks,
                compare_op=ALU.is_ge,
                fill=0.0,
                base=-128,
                pattern=[[-256, 2], [1, QB]],
                channel_multiplier=-1,
            )
            # identity for PE transposes
            nc.gpsimd.memset(ident, 0.0)
            nc.gpsimd.affine_select(
                out=ident,
                in_=ident,
                compare_op=ALU.not_equal,
                fill=1.0,
                base=0,
                pattern=[[-1, 128]],
                channel_multiplier=1,
            )
            bvb_r = bvb_sb.rearrange("p (h d) -> p h d", d=DH)

            def emit_qkv_group(qb2, xt_b, qt_b, kind, idx):
                """One psum accumulation group of a QKV projection phase.

                kind 'q'/'k': output j-tile idx of Q_T/K_T; kind 'v': seq
                chunk idx of V."""
                qs2 = slice(qb2 * QB, (qb2 + 1) * QB)
                ps = ps_mm.tile([128, QB], F32, tag="mm")
                if kind in ("q", "k"):
                    w_sb, b_sb = (w_q, bq_sb) if kind == "q" else (w_k, bk_sb)
                    jt = idx
                    for mt in range(8):
                        nc.tensor.matmul(
                            ps,
                            w_sb[:, mt, jt * 128 : (jt + 1) * 128],
                            xt_b[:, mt, :],
                            start=(mt == 0),
                            stop=(mt == 7),
                        )
                    dst = qt_b[:, jt, :] if kind == "q" else kt_sb[:, jt, qs2]
                    nc.vector.tensor_scalar_add(dst, ps, b_sb[:, jt : jt + 1])
                else:
                    kc = idx
                    for mt in range(8):
                        nc.tensor.matmul(
                            ps,
                            xt_b[:, mt, kc * 128 : (kc + 1) * 128],
                            w_v[:, mt, :],
                            start=(mt == 0),
                            stop=(mt == 7),
                        )
                    with nc.allow_low_precision(
                        reason="V in bf16 feeds bf16 attnV matmuls; ~0.4%"
                        " rel err is within the 2e-2 tolerance"
                    ):
                        nc.vector.tensor_tensor(
                            v_sb[:, qb2 * 4 + kc, :, 0:DH],
                            ps.rearrange("p (h d) -> p h d", d=DH),
                            bvb_r,
                            ALU.add,
                        )

            GROUPS = [("q", i) for i in range(4)] + [("k", i) for i in range(4)] + [
                ("v", i) for i in range(4)
            ]

            def make_proj_group(qb2, ao_t_b, et, on_act=False):
                qs2 = slice(qb2 * QB, (qb2 + 1) * QB)

                def emit():
                    ps = ps_mm.tile([128, QB], F32, tag="mm")
                    for ct in range(4):
                        nc.tensor.matmul(
                            ps,
                            w_o[:, ct, et * 128 : (et + 1) * 128],
                            ao_t_b[:, ct, :],
                            start=(ct == 0),
                            stop=(ct == 3),
                        )
                    y_t = yp.tile([128, QB], F32, tag="y")
                    if on_act:
                        nc.scalar.activation(y_t, ps, AF.Copy)
                    else:
                        nc.vector.tensor_copy(y_t, ps)
                    nc.sync.dma_start(yt[et * 128 : (et + 1) * 128, qs2], y_t)

                return emit

            def make_tr_group(ao_b, ao_t_b, hp, base):
                """Transpose a pair's AO [q, 2x64] sub-tiles back to
                AO_T[c, q] for the output projection. Rides the ps_mm
                pool (f32 transpose) to stay within the 8 psum banks."""

                def emit():
                    ps_t = ps_mm.tile([128, QB], F32, tag="mm")
                    ps_tv = ps_t.rearrange("p (a q) -> p a q", q=128)
                    for qsub in range(2):
                        nc.tensor.transpose(
                            ps_tv[:, qsub, :], ao_b[:, qsub], ident
                        )
                    with nc.allow_low_precision(
                        reason="AO_T in bf16 feeds bf16 projection matmuls;"
                        " ~0.4% rel err is within the 2e-2 tolerance"
                    ):
                        nc.vector.tensor_copy(
                            ao_t_b[:, hp, base : base + AB],
                            ps_t[:, 0:AB],
                        )

                return emit

            workq = []  # FIFO: transpose groups, then projection groups
            late_projq = []  # proj groups held back for the filler-poor tail
            next_q = []  # next 512-block's QKV group closures
            defer_q = []  # v-tail of next block's QKV, run in its own block

            # q-block 0 projections up front
            region("qkv0")
            qt_blk = qtp.tile([128, 4, QB], MDT, tag="qt")
            for kind, idx in GROUPS:
                emit_qkv_group(0, xt_cur, qt_blk, kind, idx)
            xt_blk = xt_cur
            ao_t_blk = atp.tile([128, 4, QB], BF, tag="aot")

            for qb in range(NQB):
                # stage next q-block: x prefetch + Q_T tile; its 12
                # projection groups are woven between attention steps below.
                # The v kc2/kc3 tail (k-tiles 4qb+6, 4qb+7) is deferred into
                # the next block itself — its first sub-block doesn't need
                # them, and the late blocks are filler-poor.
                if qb + 1 < NQB:
                    xt_next = xtp.tile([128, 8, QB], MDT, tag="xt")
                    nqs = slice((qb + 1) * QB, (qb + 2) * QB)
                    for mt in range(8):
                        nc.sync.dma_start(xt_next[:, mt, :], xt_r[:, mt, nqs])
                    qt_next = qtp.tile([128, 4, QB], MDT, tag="qt")
                    ao_t_next = atp.tile([128, 4, QB], BF, tag="aot")
                    mk = (
                        lambda k, i, q2=qb + 1, xn=xt_next, qn=qt_next: (
                            lambda: emit_qkv_group(q2, xn, qn, k, i)
                        )
                    )
                    next_q = [mk(k, i) for k, i in GROUPS[:10]]
                    defer_next = [mk(k, i) for k, i in GROUPS[10:]]
                else:
                    xt_next = qt_next = ao_t_next = None
                    next_q = []
                    defer_next = []
                if qb == 0:
                    # Wo is first needed by proj0, well after qb1's x
                    # prefetch — keep it behind that in the load queue
                    wo_r = wot.rearrange("(ct p) e -> p ct e", p=128)
                    for ct in range(4):
                        nc.sync.dma_start(w_o[:, ct, :], wo_r[:, ct, :])

                for sub in range(2):
                    a = 2 * qb + sub  # attention block index (256 tokens)
                    qoff = sub * AB
                    region(f"attn{a}")
                    if sub == 1:
                        # v kc2/kc3 of THIS block must be in flight before
                        # its second sub-block (k-tiles 4qb+2, 4qb+3)
                        while defer_q:
                            defer_q.pop(0)()
                    for hp in range(4):
                        # per-pair filler budget: deferred v-groups and
                        # transposes/projections first (FIFO), then next
                        # block's QKV groups, then held-back projections
                        filler = []
                        for _ in range(1):
                            if defer_q:
                                filler.append(defer_q.pop(0))
                        for _ in range(2):
                            if workq:
                                filler.append(workq.pop(0))
                        for _ in range(3):
                            if next_q:
                                filler.append(next_q.pop(0))
                            elif workq:
                                filler.append(workq.pop(0))
                            elif late_projq:
                                filler.append(late_projq.pop(0))

                        ps_av = ps_av_pool.tile([128, 2, 2, DH + 1], F32, tag="av")

                        def make_av(p2t, kt2, a=a, hp=hp, ps_av=ps_av):
                            # flipped attnV for both k-tiles of step kt2,
                            # per live 128-wide q sub-tile
                            def emit():
                                for kte in range(2):
                                    kt = 2 * kt2 + kte
                                    for qsub in range(2):
                                        if 2 * a + qsub < kt:
                                            continue  # above the diagonal
                                        for h in range(2):
                                            nc.tensor.matmul(
                                                ps_av[:, qsub, h, :],
                                                p2t[:, kte, h,
                                                    qsub * 128 : qsub * 128 + 128],
                                                v_sb[:, kt, 2 * hp + h, :],
                                                start=(kt == 0),
                                                stop=(kt == 2 * a + qsub),
                                            )

                            return emit

                        pend_av = None
                        for kt2 in range(a + 1):
                            ps_sc = ps_s_pool.tile([128, 2, 2, AB], F32, tag="s")
                            for kte in range(2):
                                kt = 2 * kt2 + kte
                                kts = slice(kt * 128, (kt + 1) * 128)
                                for h in range(2):
                                    hs = slice(h * 64, h * 64 + 64)
                                    nc.tensor.matmul(
                                        ps_sc[:, kte, h, :],
                                        kt_sb[hs, hp, kts],
                                        qt_blk[hs, hp, qoff : qoff + AB],
                                        start=True,
                                        stop=True,
                                    )
                            p2 = p2p.tile([128, 2, 2, AB], BF, tag="p")
                            if kt2 < a:
                                # one exp covers both k-tiles x both heads
                                nc.scalar.activation(p2, ps_sc, AF.Exp, scale=SCALE)
                            else:
                                # diagonal step: split so the even k-tile's
                                # probabilities are ready sooner; skip the odd
                                # k-tile's fully-dead first half; mask the
                                # 128-wide band with the band-local triangle
                                nc.scalar.activation(
                                    p2[:, 0], ps_sc[:, 0], AF.Exp, scale=SCALE
                                )
                                nc.vector.tensor_tensor(
                                    p2[:, 0, :, 0:128],
                                    p2[:, 0, :, 0:128],
                                    masks[:, 0, None, 128:256].to_broadcast(
                                        (128, 2, 128)
                                    ),
                                    ALU.mult,
                                )
                                nc.scalar.activation(
                                    p2[:, 1, :, 128:AB],
                                    ps_sc[:, 1, :, 128:AB],
                                    AF.Exp,
                                    scale=SCALE,
                                )
                                nc.vector.tensor_tensor(
                                    p2[:, 1, :, 128:AB],
                                    p2[:, 1, :, 128:AB],
                                    masks[:, 0, None, 128:256].to_broadcast(
                                        (128, 2, 128)
                                    ),
                                    ALU.mult,
                                )
                            # 1-step software pipeline: the PREVIOUS step's
                            # attnV goes behind this step's scores so its exp
                            # latency hides under real PE work
                            if pend_av is not None:
                                pend_av()
                            if kt2 % 2 == 1 and filler:
                                filler.pop(0)()
                            pend_av = make_av(p2, kt2)
                        pend_av()

                        # softmax denominators live in psum column 64 per q
                        # partition: reciprocal + per-partition scale
                        r2 = rp.tile([128, 2, 2, 1], F32, tag="r2")
                        nc.vector.reciprocal(r2, ps_av[:, :, :, DH : DH + 1])
                        if filler:
                            filler.pop(0)()
                        ao_b = aop.tile([128, 2, 2, DH], F32, tag="ao")
                        for qsub in range(2):
                            for h in range(2):
                                nc.vector.tensor_scalar_mul(
                                    ao_b[:, qsub, h, :],
                                    ps_av[:, qsub, h, 0:DH],
                                    r2[:, qsub, h, :],
                                )
                        workq.append(
                            make_tr_group(ao_b, ao_t_blk, hp, qoff)
                        )
                        while filler:
                            filler.pop(0)()

                # all 8 pairs of this 512-block done: queue its projection.
                # proj(qb1) is held back for qb3, whose attention has no
                # next-block QKV work to hide ACT (exp) latency behind.
                projs = [
                    make_proj_group(qb, ao_t_blk, et, on_act=(qb == NQB - 1))
                    for et in range(8)
                ]
                if qb == 1:
                    late_projq.extend(projs)
                else:
                    workq.extend(projs)
                defer_q = defer_next
                xt_blk = xt_next
                qt_blk = qt_next
                ao_t_blk = ao_t_next

            # drain leftovers: last q-block's transposes + projections
            region("proj3")
            while late_projq:
                late_projq.pop(0)()
            while workq:
                workq.pop(0)()

    nc.compile()
    return nc


def make_in_maps(x, Wq_w, Wk_w, Wv_w, Wo_w, Wq_b, Wk_b, Wv_b):
    """Per-core host-side sharding + layout prep."""
    x = np.asarray(x, dtype=np.float32)
    in_maps = []
    for c in range(NCORES):
        b, g = divmod(c, 2)
        cols = slice(g * C, (g + 1) * C)
        in_maps.append(
            {
                "xt": np.ascontiguousarray(x[b].T),
                "wqt": np.ascontiguousarray(np.asarray(Wq_w).T[:, cols]),
                "wkt": np.ascontiguousarray(np.asarray(Wk_w).T[:, cols]),
                "wvt": np.ascontiguousarray(np.asarray(Wv_w).T[:, cols]),
                "wot": np.ascontiguousarray(np.asarray(Wo_w)[:, cols].T).astype(
                    ml_dtypes.bfloat16
                ),
                "bq": np.ascontiguousarray(
                    np.asarray(Wq_b)[cols].reshape(C // 128, 128).T
                ),
                "bk": np.ascontiguousarray(
                    np.asarray(Wk_b)[cols].reshape(C // 128, 128).T
                ),
                "bvb": np.ascontiguousarray(
                    np.tile(np.asarray(Wv_b)[cols][None, :], (128, 1))
                ),
            }
        )
    return in_maps


_NC_CACHE = {}
last_results = None  # test harness reads profiling info from here


def kernel(x, mask, Wq_w, Wq_b, Wk_w, Wk_b, Wv_w, Wv_b, Wo_w, Wo_b):
    global last_results
    if "nc" not in _NC_CACHE:
        _NC_CACHE["nc"] = build_nc()
    nc = _NC_CACHE["nc"]

    in_maps = make_in_maps(x, Wq_w, Wk_w, Wv_w, Wo_w, Wq_b, Wk_b, Wv_b)
    res = run_bass_kernel_spmd(nc, in_maps, list(range(NCORES)))
    last_results = res

    bo = np.asarray(Wo_b, dtype=np.float32)
    y = np.empty((B, S, D), dtype=np.float32)
    for b in range(B):
        yt = res.results[2 * b]["yt"] + res.results[2 * b + 1]["yt"]
        y[b] = yt.T + bo[None, :]
    return y


# revision 16
# speedup vs baseline: 1.1581x; 1.0468x over previous
"""Trainium2 Bass kernel for causal multi-head attention.

Problem: B=4, S=2048, D=1024, H=16 heads, Dh=64, fp32, causal mask.
Sharding: 8 cores = 4 batches x 2 head-groups (8 heads each). No
collectives: each core produces a partial output projection y_T
[1024, 2048] for its batch; the host sums the two head-group partials
per batch and adds the output bias.

Device-side design (per core):
  - QKV projections in fp32r (full PE rate at moving dim >= 256):
      Q_T, K_T = W_slice^T.T @ X_T      (lhsT = W^T slices, rhs = X_T)
      V natural [token, feature]        (lhsT = X_T chunk, rhs = W_v^T)
    V is stored bf16 with a 65th ones-column per head.
  - scores transposed per head: S_T[k, q] = K_T_h.T @ Q_T_h (contraction
    over Dh=64 partitions), fp32r, in 256-wide q sub-blocks; exp on ACT
    (scale folded) emits bf16 probabilities p2.
  - attnV FLIPPED: out[q_part, dh] = p2_slice.T @ V_tile — moving dim is
    only 65 (64 V cols + ones col), vs 512 in the [dh, q] orientation,
    halving attnV PE cycles (matmul cost is moving-dim rows only; output
    partitions are free). Requires bf16 operands (fp32r is 1/4 rate
    below 256-wide moving). The ones column accumulates the softmax
    denominator per q PARTITION, so the division is a cheap per-partition
    DVE tensor_scalar instead of ones-matmul broadcasts.
  - AO[q, c] is transposed back to AO_T[c, q] with bf16 PE transposes
    (128 rows each) to feed the output projection (contraction over c
    needs c on partitions); projection runs in bf16 (same PE rate).
  - causal: dead k-tiles skipped per 128-wide q sub-tile; only the
    128-wide diagonal band is masked with one DVE multiply per k-tile
    (band-local f>=p triangle identical for every diagonal tile).
  - cross-phase software pipelining: next q-block's QKV groups, previous
    blocks' projection groups, and AO transposes are woven between
    attention steps so the in-order PE stream always has independent
    work during softmax (ACT) dependency stalls.
"""

import numpy as np
import ml_dtypes

import concourse.tile as tile
from concourse import bacc, mybir
from concourse.bass_utils import run_bass_kernel_spmd

B = 4
S = 2048
D = 1024
H = 16
DH = 64
NCORES = 8
HPC = 8  # heads per core
C = HPC * DH  # 512 local channels per core
QB = 512  # QKV/projection block (matmul moving free dim)
AB = 256  # attention q sub-block
NQB = S // QB  # 4
NAB = S // AB  # 8
NKT = S // 128  # 16 k-tiles
SCALE = 1.0 / float(np.sqrt(DH))

F32 = mybir.dt.float32
BF = mybir.dt.bfloat16
AF = mybir.ActivationFunctionType
ALU = mybir.AluOpType


def build_nc():
    """Build the single-core Bass program (SPMD-replicated on 8 cores)."""
    MDT = mybir.dt.float32r

    nc = bacc.Bacc("TRN2", target_bir_lowering=False, debug=False)
    regions = []
    nc._regions = regions

    def region(name):
        regions.append((name, len(nc.inst_map)))

    xt = nc.dram_tensor("xt", [D, S], MDT, kind="ExternalInput").ap()
    wqt = nc.dram_tensor("wqt", [D, C], MDT, kind="ExternalInput").ap()
    wkt = nc.dram_tensor("wkt", [D, C], MDT, kind="ExternalInput").ap()
    wvt = nc.dram_tensor("wvt", [D, C], MDT, kind="ExternalInput").ap()
    wot = nc.dram_tensor("wot", [C, D], BF, kind="ExternalInput").ap()
    bq_d = nc.dram_tensor("bq", [128, C // 128], F32, kind="ExternalInput").ap()
    bk_d = nc.dram_tensor("bk", [128, C // 128], F32, kind="ExternalInput").ap()
    bvb_d = nc.dram_tensor("bvb", [128, C], F32, kind="ExternalInput").ap()
    yt = nc.dram_tensor("yt", [D, S], F32, kind="ExternalOutput").ap()

    xt_r = xt.rearrange("(mt p) s -> p mt s", p=128)

    with tile.TileContext(nc) as tc:
        with (
            tc.tile_pool(name="singles", bufs=1) as singles,
            tc.tile_pool(name="xtp", bufs=2) as xtp,
            tc.tile_pool(name="qtp", bufs=2) as qtp,
            tc.tile_pool(name="p2p", bufs=3) as p2p,
            tc.tile_pool(name="aop", bufs=4) as aop,
            tc.tile_pool(name="atp", bufs=3) as atp,
            tc.tile_pool(name="rp", bufs=2) as rp,
            tc.tile_pool(name="yp", bufs=4) as yp,
            tc.tile_pool(name="ps_mm", bufs=2, space="PSUM") as ps_mm,
            tc.tile_pool(name="ps_s", bufs=2, space="PSUM") as ps_s_pool,
            tc.tile_pool(name="ps_av", bufs=2, space="PSUM") as ps_av_pool,
        ):
            # ---- persistent tiles -------------------------------------
            w_q = singles.tile([128, 8, C], MDT, tag="w_q")
            w_k = singles.tile([128, 8, C], MDT, tag="w_k")
            w_v = singles.tile([128, 8, C], MDT, tag="w_v")
            w_o = singles.tile([128, 4, D], BF, tag="w_o")
            bq_sb = singles.tile([128, C // 128], F32, tag="bq")
            bk_sb = singles.tile([128, C // 128], F32, tag="bk")
            bvb_sb = singles.tile([128, C], F32, tag="bvb")
            kt_sb = singles.tile([128, 4, S], MDT, tag="kt")
            v_sb = singles.tile([128, NKT, HPC, DH + 1], BF, tag="v")
            masks = singles.tile([128, 2, QB], BF, tag="masks")
            ident = singles.tile([128, 128], F32, tag="ident")

            # first x block goes ahead of everything so PE unblocks ASAP;
            # wq follows j-tile-major so the first Q group only needs the
            # jt=0 quarter of wq
            xt_cur = xtp.tile([128, 8, QB], MDT, tag="xt")
            wq_r = wqt.rearrange("(mt p) j -> p mt j", p=128)
            wk_r = wkt.rearrange("(mt p) j -> p mt j", p=128)
            wv_r = wvt.rearrange("(mt p) j -> p mt j", p=128)
            # split the preload across BOTH hwdge queues: x + wq on the SP
            # queue, wk + wv on the (early-idle) ACT queue — halves the
            # serial DMA-issue latency in front of wv
            for mt in range(8):
                nc.sync.dma_start(xt_cur[:, mt, :], xt_r[:, mt, 0:QB])
                nc.sync.dma_start(w_q[:, mt, :], wq_r[:, mt, :])
            for mt in range(8):
                nc.scalar.dma_start(w_k[:, mt, :], wk_r[:, mt, :])
            for mt in range(8):
                nc.scalar.dma_start(w_v[:, mt, :], wv_r[:, mt, :])
            # small/constant inputs ride the idle gpsimd (SWDGE) queue
            nc.gpsimd.dma_start(bq_sb, bq_d)
            nc.gpsimd.dma_start(bk_sb, bk_d)
            nc.gpsimd.dma_start(bvb_sb, bvb_d)

            # ones column (65th) of every per-head V block; bf16 memsets
            # are encodable so no host ones tensor is needed
            nc.vector.memset(v_sb[:, :, :, DH : DH + 1], 1.0)
            # mask tile; only the [128:256] slice of row 0 is used — in
            # band-local coordinates it is the f>=p triangle that every
            # diagonal tile needs
            nc.vector.memset(masks, 1.0)
            # warm-up matmuls on the freshly-memset mask tile: they depend
            # only on the early DVE memset, so they execute during the
            # initial DMA wait and keep the PE activity window warm
            for _ in range(5):
                ps_w = ps_mm.tile([128, QB], F32, tag="mm")
                nc.tensor.matmul(
                    ps_w, masks[:, 0, 0:128], masks[:, 1, :], start=True, stop=True
                )
            nc.gpsimd.affine_select(
                out=masks,
                in_=masks,
                compare_op=ALU.is_ge,
                fill=0.0,
                base=-128,
                pattern=[[-256, 2], [1, QB]],
                channel_multiplier=-1,
            )
            # identity for PE transposes
            nc.gpsimd.memset(ident, 0.0)
            nc.gpsimd.affine_select(
                out=ident,
                in_=ident,
                compare_op=ALU.not_equal,
                fill=1.0,
                base=0,
                pattern=[[-1, 128]],
                channel_multiplier=1,
            )
            bvb_r = bvb_sb.rearrange("p (h d) -> p h d", d=DH)

            def emit_qkv_group(qb2, xt_b, qt_b, kind, idx):
                """One psum accumulation group of a QKV projection phase.

                kind 'q'/'k': output j-tile idx of Q_T/K_T; kind 'v': seq
                chunk idx of V."""
                qs2 = slice(qb2 * QB, (qb2 + 1) * QB)
                ps = ps_mm.tile([128, QB], F32, tag="mm")
                if kind in ("q", "k"):
                    w_sb, b_sb = (w_q, bq_sb) if kind == "q" else (w_k, bk_sb)
                    jt = idx
                    for mt in range(8):
                        nc.tensor.matmul(
                            ps,
                            w_sb[:, mt, jt * 128 : (jt + 1) * 128],
                            xt_b[:, mt, :],
                            start=(mt == 0),
                            stop=(mt == 7),
                        )
                    dst = qt_b[:, jt, :] if kind == "q" else kt_sb[:, jt, qs2]
                    nc.vector.tensor_scalar_add(dst, ps, b_sb[:, jt : jt + 1])
                else:
                    kc = idx
                    for mt in range(8):
                        nc.tensor.matmul(
                            ps,
                            xt_b[:, mt, kc * 128 : (kc + 1) * 128],
                            w_v[:, mt, :],
                            start=(mt == 0),
                            stop=(mt == 7),
                        )
                    with nc.allow_low_precision(
                        reason="V in bf16 feeds bf16 attnV matmuls; ~0.4%"
                        " rel err is within the 2e-2 tolerance"
                    ):
                        nc.vector.tensor_tensor(
                            v_sb[:, qb2 * 4 + kc, :, 0:DH],
                            ps.rearrange("p (h d) -> p h d", d=DH),
                            bvb_r,
                            ALU.add,
                        )

            GROUPS = [("q", i) for i in range(4)] + [("k", i) for i in range(4)] + [
                ("v", i) for i in range(4)
            ]

            def make_proj_group(qb2, ao_t_b, et, on_act=False):
                qs2 = slice(qb2 * QB, (qb2 + 1) * QB)

                def emit():
                    ps = ps_mm.tile([128, QB], F32, tag="mm")
                    for ct in range(4):
                        nc.tensor.matmul(
                            ps,
                            w_o[:, ct, et * 128 : (et + 1) * 128],
                            ao_t_b[:, ct, :],
                            start=(ct == 0),
                            stop=(ct == 3),
                        )
                    y_t = yp.tile([128, QB], F32, tag="y")
                    if on_act:
                        nc.scalar.activation(y_t, ps, AF.Copy)
                    else:
                        nc.vector.tensor_copy(y_t, ps)
                    nc.sync.dma_start(yt[et * 128 : (et + 1) * 128, qs2], y_t)

                return emit

            def make_tr_group(ao_b, ao_t_b, hp, base):
                """Transpose a pair's AO [q, 2x64] sub-tiles back to
                AO_T[c, q] for the output projection. Rides the ps_mm
                pool (f32 transpose) to stay within the 8 psum banks."""

                def emit():
                    ps_t = ps_mm.tile([128, QB], F32, tag="mm")
                    ps_tv = ps_t.rearrange("p (a q) -> p a q", q=128)
                    for qsub in range(2):
                        nc.tensor.transpose(
                            ps_tv[:, qsub, :], ao_b[:, qsub], ident
                        )
                    with nc.allow_low_precision(
                        reason="AO_T in bf16 feeds bf16 projection matmuls;"
                        " ~0.4% rel err is within the 2e-2 tolerance"
                    ):
                        nc.vector.tensor_copy(
                            ao_t_b[:, hp, base : base + AB],
                            ps_t[:, 0:AB],
                        )

                return emit

            workq = []  # FIFO: transpose groups, then projection groups
            late_projq = []  # proj groups held back for the filler-poor tail
            next_q = []  # next 512-block's QKV group closures
            defer_q = []  # v-tail of next block's QKV, run in its own block

            # q-block 0 projections up front
            region("qkv0")
            qt_blk = qtp.tile([128, 4, QB], MDT, tag="qt")
            for kind, idx in GROUPS:
                emit_qkv_group(0, xt_cur, qt_blk, kind, idx)
            xt_blk = xt_cur
            ao_t_blk = atp.tile([128, 4, QB], BF, tag="aot")

            for qb in range(NQB):
                # stage next q-block: x prefetch + Q_T tile; its 12
                # projection groups are woven between attention steps below.
                # The v kc2/kc3 tail (k-tiles 4qb+6, 4qb+7) is deferred into
                # the next block itself — its first sub-block doesn't need
                # them, and the late blocks are filler-poor.
                if qb + 1 < NQB:
                    xt_next = xtp.tile([128, 8, QB], MDT, tag="xt")
                    nqs = slice((qb + 1) * QB, (qb + 2) * QB)
                    for mt in range(8):
                        nc.sync.dma_start(xt_next[:, mt, :], xt_r[:, mt, nqs])
                    qt_next = qtp.tile([128, 4, QB], MDT, tag="qt")
                    ao_t_next = atp.tile([128, 4, QB], BF, tag="aot")
                    mk = (
                        lambda k, i, q2=qb + 1, xn=xt_next, qn=qt_next: (
                            lambda: emit_qkv_group(q2, xn, qn, k, i)
                        )
                    )
                    next_q = [mk(k, i) for k, i in GROUPS[:10]]
                    defer_next = [mk(k, i) for k, i in GROUPS[10:]]
                else:
                    xt_next = qt_next = ao_t_next = None
                    next_q = []
                    defer_next = []
                if qb == 0:
                    # Wo is first needed by proj0, well after the K/V
                    # weights — ride the ACT queue behind them
                    wo_r = wot.rearrange("(ct p) e -> p ct e", p=128)
                    for ct in range(4):
                        nc.scalar.dma_start(w_o[:, ct, :], wo_r[:, ct, :])

                for sub in range(2):
                    a = 2 * qb + sub  # attention block index (256 tokens)
                    qoff = sub * AB
                    region(f"attn{a}")
                    if sub == 1:
                        # v kc2/kc3 of THIS block must be in flight before
                        # its second sub-block (k-tiles 4qb+2, 4qb+3)
                        while defer_q:
                            defer_q.pop(0)()
                    for hp in range(4):
                        # per-pair filler budget: deferred v-groups and
                        # transposes/projections first (FIFO), then next
                        # block's QKV groups, then held-back projections
                        filler = []
                        for _ in range(1):
                            if defer_q:
                                filler.append(defer_q.pop(0))
                        for _ in range(2):
                            if workq:
                                filler.append(workq.pop(0))
                        for _ in range(3):
                            if next_q:
                                filler.append(next_q.pop(0))
                            elif workq:
                                filler.append(workq.pop(0))
                            elif late_projq:
                                filler.append(late_projq.pop(0))

                        ps_av = ps_av_pool.tile([128, 2, 2, DH + 1], F32, tag="av")

                        def make_av(p2t, kt2, a=a, hp=hp, ps_av=ps_av):
                            # flipped attnV for both k-tiles of step kt2,
                            # per live 128-wide q sub-tile
                            def emit():
                                for kte in range(2):
                                    kt = 2 * kt2 + kte
                                    for qsub in range(2):
                                        if 2 * a + qsub < kt:
                                            continue  # above the diagonal
                                        for h in range(2):
                                            nc.tensor.matmul(
                                                ps_av[:, qsub, h, :],
                                                p2t[:, kte, h,
                                                    qsub * 128 : qsub * 128 + 128],
                                                v_sb[:, kt, 2 * hp + h, :],
                                                start=(kt == 0),
                                                stop=(kt == 2 * a + qsub),
                                            )

                            return emit

                        pend_av = None
                        for kt2 in range(a + 1):
                            ps_sc = ps_s_pool.tile([128, 2, 2, AB], F32, tag="s")
                            for kte in range(2):
                                kt = 2 * kt2 + kte
                                kts = slice(kt * 128, (kt + 1) * 128)
                                for h in range(2):
                                    hs = slice(h * 64, h * 64 + 64)
                                    nc.tensor.matmul(
                                        ps_sc[:, kte, h, :],
                                        kt_sb[hs, hp, kts],
                                        qt_blk[hs, hp, qoff : qoff + AB],
                                        start=True,
                                        stop=True,
                                    )
                            p2 = p2p.tile([128, 2, 2, AB], BF, tag="p")
                            if kt2 < a:
                                # one exp covers both k-tiles x both heads
                                nc.scalar.activation(p2, ps_sc, AF.Exp, scale=SCALE)
                            else:
                                # diagonal step: split so the even k-tile's
                                # probabilities are ready sooner; skip the odd
                                # k-tile's fully-dead first half; mask the
                                # 128-wide band with the band-local triangle
                                nc.scalar.activation(
                                    p2[:, 0], ps_sc[:, 0], AF.Exp, scale=SCALE
                                )
                                nc.vector.tensor_tensor(
                                    p2[:, 0, :, 0:128],
                                    p2[:, 0, :, 0:128],
                                    masks[:, 0, None, 128:256].to_broadcast(
                                        (128, 2, 128)
                                    ),
                                    ALU.mult,
                                )
                                nc.scalar.activation(
                                    p2[:, 1, :, 128:AB],
                                    ps_sc[:, 1, :, 128:AB],
                                    AF.Exp,
                                    scale=SCALE,
                                )
                                nc.vector.tensor_tensor(
                                    p2[:, 1, :, 128:AB],
                                    p2[:, 1, :, 128:AB],
                                    masks[:, 0, None, 128:256].to_broadcast(
                                        (128, 2, 128)
                                    ),
                                    ALU.mult,
                                )
                            # 1-step software pipeline: the PREVIOUS step's
                            # attnV goes behind this step's scores so its exp
                            # latency hides under real PE work
                            if pend_av is not None:
                                pend_av()
                            if kt2 % 2 == 1 and filler:
                                filler.pop(0)()
                            pend_av = make_av(p2, kt2)
                        pend_av()

                        # softmax denominators live in psum column 64 per q
                        # partition: reciprocal + per-partition scale
                        r2 = rp.tile([128, 2, 2, 1], F32, tag="r2")
                        nc.vector.reciprocal(r2, ps_av[:, :, :, DH : DH + 1])
                        if filler:
                            filler.pop(0)()
                        ao_b = aop.tile([128, 2, 2, DH], F32, tag="ao")
                        for qsub in range(2):
                            for h in range(2):
                                nc.vector.tensor_scalar_mul(
                                    ao_b[:, qsub, h, :],
                                    ps_av[:, qsub, h, 0:DH],
                                    r2[:, qsub, h, :],
                                )
                        workq.append(
                            make_tr_group(ao_b, ao_t_blk, hp, qoff)
                        )
                        while filler:
                            filler.pop(0)()

                # all 8 pairs of this 512-block done: queue its projection.
                # proj(qb1) is held back for qb3, whose attention has no
                # next-block QKV work to hide ACT (exp) latency behind.
                projs = [
                    make_proj_group(qb, ao_t_blk, et, on_act=(qb == NQB - 1))
                    for et in range(8)
                ]
                if qb == 1:
                    late_projq.extend(projs)
                else:
                    workq.extend(projs)
                defer_q = defer_next
                xt_blk = xt_next
                qt_blk = qt_next
                ao_t_blk = ao_t_next

            # drain leftovers: last q-block's transposes + projections
            region("proj3")
            while late_projq:
                late_projq.pop(0)()
            while workq:
                workq.pop(0)()

    nc.compile()
    return nc


def make_in_maps(x, Wq_w, Wk_w, Wv_w, Wo_w, Wq_b, Wk_b, Wv_b):
    """Per-core host-side sharding + layout prep."""
    x = np.asarray(x, dtype=np.float32)
    in_maps = []
    for c in range(NCORES):
        b, g = divmod(c, 2)
        cols = slice(g * C, (g + 1) * C)
        in_maps.append(
            {
                "xt": np.ascontiguousarray(x[b].T),
                "wqt": np.ascontiguousarray(np.asarray(Wq_w).T[:, cols]),
                "wkt": np.ascontiguousarray(np.asarray(Wk_w).T[:, cols]),
                "wvt": np.ascontiguousarray(np.asarray(Wv_w).T[:, cols]),
                "wot": np.ascontiguousarray(np.asarray(Wo_w)[:, cols].T).astype(
                    ml_dtypes.bfloat16
                ),
                "bq": np.ascontiguousarray(
                    np.asarray(Wq_b)[cols].reshape(C // 128, 128).T
                ),
                "bk": np.ascontiguousarray(
                    np.asarray(Wk_b)[cols].reshape(C // 128, 128).T
                ),
                "bvb": np.ascontiguousarray(
                    np.tile(np.asarray(Wv_b)[cols][None, :], (128, 1))
                ),
            }
        )
    return in_maps


_NC_CACHE = {}
last_results = None  # test harness reads profiling info from here


def kernel(x, mask, Wq_w, Wq_b, Wk_w, Wk_b, Wv_w, Wv_b, Wo_w, Wo_b):
    global last_results
    if "nc" not in _NC_CACHE:
        _NC_CACHE["nc"] = build_nc()
    nc = _NC_CACHE["nc"]

    in_maps = make_in_maps(x, Wq_w, Wk_w, Wv_w, Wo_w, Wq_b, Wk_b, Wv_b)
    res = run_bass_kernel_spmd(nc, in_maps, list(range(NCORES)))
    last_results = res

    bo = np.asarray(Wo_b, dtype=np.float32)
    y = np.empty((B, S, D), dtype=np.float32)
    for b in range(B):
        yt = res.results[2 * b]["yt"] + res.results[2 * b + 1]["yt"]
        y[b] = yt.T + bo[None, :]
    return y
